# revision 30
# baseline (speedup 1.0000x reference)
"""Trainium2 Bass kernel for a ViT-style dense transformer block (v2, fp8).

Reference computation:
    xn = LN1(x); qkv = xn @ Wqkv.T + b; 16-head softmax attention;
    x = x + attn_out @ Wp.T + bp;
    out = x + gelu(LN2(x) @ W1.T + b1) @ W2.T + b2
Shapes: x [16, 577, 1024], heads 16, head_dim 64, hidden 4096.

Sharding: data-parallel over batch across 8 NeuronCores; each core gets 2
batch items concatenated along tokens (T = 2*577 = 1154). No collectives.

v2/v3 changes vs the bf16 baseline (742us -> ~581us measured):
  * qkv / v / proj / fc1 matmuls run in fp8(e4m3) DoubleRow mode
    (2 contraction chunks per instruction, ~1.4x tensor throughput); fc2
    stays bf16 (fp8 there would blow the 2e-2 error budget - fc1+fc2
    together measured 2.4e-2 in simulation, fc1 alone 1.6e-2, qkv/proj
    each < 2e-3).  Attention scores and attn@v run in fp8 normal mode
    (DoubleRow with the odd 65-partition output of attn@v produced NaNs
    on hardware; normal-mode fp8 is safe).
  * Activations for those matmuls are stored as fp8 with power-of-two
    scales (xn*32, q/k*32, v*32, probs, y*64); scales are folded into the
    PSUM-evacuation activation scale and the ones/bias rank-1 rows.
  * Attention softmax: the denominator comes from an appended ones-row in
    v_aug (as before) but is evacuated together with y in a single [65, qw]
    copy; the 4 sums rows of a head-pair group are gathered by GPSIMD into
    one 32-row-aligned tile, inverted with reciprocal_approx_fast (one DVE
    op instead of an 8-cycle/element exact reciprocal), broadcast by a
    rank-1 matmul of a 64-valued column (folding the y fp8 scale), and
    multiplied in.
  * LayerNorm: squares and PSUM->SBUF row copies moved to the scalar
    engine, the x*g*rstd pass split between GPSIMD and DVE, rstd via
    reciprocal_approx_fast. LN2 is emitted per token-tile interleaved with
    the proj evacuations so fc1 starts while proj finishes.
"""

import numpy as np
import ml_dtypes
from contextlib import ExitStack

import concourse.bacc as bacc
import concourse.mybir as mybir
import concourse.tile as tile
from concourse.bass_utils import run_bass_kernel_spmd

F32 = mybir.dt.float32
BF16 = mybir.dt.bfloat16
F8 = mybir.dt.float8e4
AF = mybir.ActivationFunctionType
ALU = mybir.AluOpType
DR = mybir.MatmulPerfMode.DoubleRow

N_CORES = 8
P = 128
C = 1024
KC = C // P          # 8 feature chunks
H = 16               # heads
D = 64               # head dim
HID = 4096
MH = HID // P        # 32 hidden chunks
NTOK = 577           # tokens per item
T = 2 * NTOK         # tokens per core
TPAD = 1280          # padded fp8 activation free dim (%16, covers v overread)
EXPAD = 592          # padded exp-tile free dim (%16)
EPS = 1e-5

SX = 32.0            # fp8 scale: xn / xs (LN outputs)
SW = 1024.0          # fp8 scale: weights
SQ = 32.0            # fp8 scale: q / k
SV = 32.0            # fp8 scale: v (also the ones-row value in v_aug)
SY = 64.0            # fp8 scale: yT (bcast column value)

F8_FC1 = True        # fc1 in fp8 DoubleRow (rel-err budget allows one fc)

# token tiles for dense (non-attention) phases; <=512 for one PSUM bank
TT = [(0, 512), (512, 512), (1024, 130)]
# query tiles per item (relative to item start)
QT = [(0, 512), (512, 65)]
# key chunks per item: 4 full 128s + one 65
KCH = [(0, 128), (128, 128), (256, 128), (384, 128), (512, 65)]

LAST_EXEC_NS = None
_NC = None


def _build():
    nc = bacc.Bacc(
        "TRN2", target_bir_lowering=False, debug=False, num_devices=N_CORES
    )

    xT_d = nc.dram_tensor("xT", [P, KC, T], F32, kind="ExternalInput")
    wqk_d = nc.dram_tensor("wqk", [2 * KC, P, KC, P], F8, kind="ExternalInput")
    wv_d = nc.dram_tensor("wv", [P, KC, C], F8, kind="ExternalInput")
    wp_d = nc.dram_tensor("wp", [P, KC, C], F8, kind="ExternalInput")
    w1_d = nc.dram_tensor(
        "w1", [MH, P, KC, P], F8 if F8_FC1 else BF16, kind="ExternalInput"
    )
    w2_d = nc.dram_tensor("w2", [KC, P, MH, P], BF16, kind="ExternalInput")
    qkb_d = nc.dram_tensor("qkb", [P, 2 * KC], F32, kind="ExternalInput")
    bv_d = nc.dram_tensor("bv", [1, C], BF16, kind="ExternalInput")
    pb_d = nc.dram_tensor("pb", [P, KC], F32, kind="ExternalInput")
    b1_d = nc.dram_tensor("b1", [P, MH], F32, kind="ExternalInput")
    b2_d = nc.dram_tensor("b2", [P, KC], F32, kind="ExternalInput")
    gb1_d = nc.dram_tensor("gb1", [2, KC, P], BF16, kind="ExternalInput")
    gb2_d = nc.dram_tensor("gb2", [2, KC, P], BF16, kind="ExternalInput")
    g1c_d = nc.dram_tensor("g1c", [P, KC], F32, kind="ExternalInput")
    g2c_d = nc.dram_tensor("g2c", [P, KC], F32, kind="ExternalInput")
    out_d = nc.dram_tensor("outT", [P, KC, T], F32, kind="ExternalOutput")

    with tile.TileContext(nc) as tc, ExitStack() as top:
        const_p = top.enter_context(tc.tile_pool(name="consts", bufs=1))
        xp = top.enter_context(tc.tile_pool(name="xres", bufs=1))
        yp = top.enter_context(tc.tile_pool(name="ypool", bufs=1))

        # residual stream, feature-major f32 (updated in place with x2);
        # x pieces go out first so LN1 tile0 isn't queued behind the consts
        xT = xp.tile([P, KC, T], F32)
        for _t0, _tw in TT:
            for _c in range(0, KC, 4):
                nc.sync.dma_start(
                    out=xT[:, _c:_c + 4, _t0:_t0 + _tw],
                    in_=xT_d[:, _c:_c + 4, _t0:_t0 + _tw],
                )

        qkb = const_p.tile([P, 2 * KC], F32)
        nc.sync.dma_start(out=qkb, in_=qkb_d[:])
        bv = const_p.tile([1, C], BF16)
        nc.sync.dma_start(out=bv, in_=bv_d[:])
        pb = const_p.tile([P, KC], F32)
        nc.sync.dma_start(out=pb, in_=pb_d[:])
        b1 = const_p.tile([P, MH], F32)
        nc.sync.dma_start(out=b1, in_=b1_d[:])
        b2 = const_p.tile([P, KC], F32)
        nc.sync.dma_start(out=b2, in_=b2_d[:])
        gb1 = const_p.tile([2, KC, P], BF16)
        nc.sync.dma_start(out=gb1, in_=gb1_d[:])
        gb2 = const_p.tile([2, KC, P], BF16)
        nc.sync.dma_start(out=gb2, in_=gb2_d[:])
        g1c = const_p.tile([P, KC], F32)
        nc.sync.dma_start(out=g1c, in_=g1c_d[:])
        g2c = const_p.tile([P, KC], F32)
        nc.sync.dma_start(out=g2c, in_=g2c_d[:])
        inv_ones = const_p.tile([P, 1], BF16)
        nc.vector.memset(inv_ones, 1.0 / C)
        ones_pp = const_p.tile([P, P], BF16)
        nc.vector.memset(ones_pp, 1.0)
        ones_r1 = ones_pp[0:1, :]
        c64 = const_p.tile([P, D], BF16)  # SY-valued col; rank-1 lhsT
        nc.vector.memset(c64, SY)

        def layernorm_tile(src, gb, g_c, dst, t0, tw, ln_sb, ln_ps):
            """dst[:, :, t0:t0+tw] (fp8) = SX * LN(src[f32]), one token tile."""
            ts = slice(t0, t0 + tw)
            rows2 = ln_sb.tile([2, 512], BF16, tag="rows2", name="rows2",
                               bufs=2)[:, :tw]
            nc.vector.memset(rows2, -1.0)  # row1 stays -1; row0 overwritten
            s1 = ln_ps.tile([1, 512], F32, tag="s1", name="s1", bufs=1)[:, :tw]
            s2 = ln_ps.tile([1, 512], F32, tag="s2", name="s2", bufs=1)[:, :tw]
            for ci in range(KC):
                xbf = ln_sb.tile([P, 512], BF16, tag="xbf", name="xbf",
                                 bufs=2)[:, :tw]
                nc.scalar.copy(xbf, src[:, ci, ts])
                xsq = ln_sb.tile([P, 512], BF16, tag="xsq", name="xsq",
                                 bufs=2)[:, :tw]
                nc.scalar.square(xsq, xbf)
                nc.tensor.matmul(
                    s1, inv_ones, xbf, start=(ci == 0), stop=(ci == KC - 1)
                )
                nc.tensor.matmul(
                    s2, inv_ones, xsq, start=(ci == 0), stop=(ci == KC - 1)
                )
            msq = ln_sb.tile([1, 512], F32, tag="msq", name="msq", bufs=1)[:, :tw]
            nc.scalar.square(msq, s1)
            veps = ln_sb.tile([1, 512], F32, tag="veps", name="veps", bufs=1)[:, :tw]
            nc.vector.scalar_tensor_tensor(
                veps, s2, EPS, msq, ALU.add, ALU.subtract
            )
            sd = ln_sb.tile([1, 512], F32, tag="sd", name="sd", bufs=1)[:, :tw]
            nc.scalar.sqrt(sd, veps)
            r32 = ln_sb.tile([1, 512], F32, tag="r32", name="r32", bufs=1)[:, :tw]
            nc.vector.reciprocal_approx_fast(out=r32, in_=sd)
            rbf = ln_sb.tile([1, 512], BF16, tag="rbf", name="rbf", bufs=1)[:, :tw]
            nc.vector.tensor_copy(out=rbf, in_=r32)
            nc.vector.tensor_mul(rows2[0:1, :], s1, r32)  # mr row
            ps_rb = ln_ps.tile([P, 512], F32, tag="lnrb", name="lnrb",
                               bufs=1)[:, :tw]
            nc.tensor.matmul(ps_rb, ones_r1, rbf, start=True, stop=True)
            rb = ln_sb.tile([P, 512], F32, tag="rb", name="rb", bufs=2)[:, :tw]
            nc.scalar.copy(rb, ps_rb)
            for ci in range(KC):
                # gmrb[c,t] = SX*(g[c]*mr[t] - b[c])
                gmrb = ln_ps.tile([P, 512], F32, tag="gmrb", name="gmrb",
                                  bufs=1)[:, :tw]
                nc.tensor.matmul(
                    gmrb, gb[:, ci, :], rows2, start=True, stop=True
                )
                t1 = ln_sb.tile([P, 512], F32, tag="t1", name="t1",
                                bufs=3)[:, :tw]
                eng = nc.gpsimd if ci < 4 else nc.vector
                eng.tensor_tensor(
                    out=t1, in0=src[:, ci, ts], in1=rb, op=ALU.mult
                )
                nc.vector.scalar_tensor_tensor(
                    dst[:, ci, ts], t1, g_c[:, ci:ci + 1], gmrb,
                    ALU.mult, ALU.subtract,
                )

        # --------------- LN1 + QKV (fp8 DoubleRow) -----------------------
        attn_scope = top.enter_context(ExitStack())
        qk_p = attn_scope.enter_context(tc.tile_pool(name="qkp", bufs=1))
        v_p = attn_scope.enter_context(tc.tile_pool(name="vp", bufs=1))
        yT8 = yp.tile([P, KC, TPAD], F8)  # attention output (pre-proj), *SY
        qkT = qk_p.tile([P, 2 * KC, T], F8)  # q,k *SQ
        v_aug = v_p.tile([P, 10, H, 65], F8)  # v *SV, ones-row = SV
        nc.vector.memset(v_aug[:, :, :, 64:65], SV)
        xn8 = None
        wqk_pool = None

        def emit_qk(mm_ps, m, tiles, psbufs=2):
            wqkb = wqk_pool.tile([P, KC, P], F8, tag="wqkb", name="wqkb")
            nc.sync.dma_start(out=wqkb, in_=wqk_d[m])
            for t0, tw in tiles:
                ts = slice(t0, t0 + tw)
                ps = mm_ps.tile(
                    [P, 512], F32, tag="qk", name="qk", bufs=psbufs
                )[:, :tw]
                for cj in range(KC // 2):
                    nc.tensor.matmul(
                        ps, wqkb[:, 2 * cj:2 * cj + 2, :],
                        xn8[:, 2 * cj:2 * cj + 2, ts],
                        start=(cj == 0), stop=(cj == KC // 2 - 1),
                        perf_mode=DR,
                    )
                nc.scalar.activation(
                    qkT[:, m, ts], ps, AF.Identity,
                    bias=qkb[:, m:m + 1], scale=SQ / (SW * SX),
                )

        def emit_v(mm_ps, wv, it):
            for c_i, (k0, kw) in enumerate(KCH):
                tok0 = it * NTOK + k0
                for half in range(2):
                    ps = mm_ps.tile([P, 512], F32, tag="v", name="v", bufs=2)
                    for cj in range(KC // 2):
                        nc.tensor.matmul(
                            ps,
                            xn8[:, 2 * cj:2 * cj + 2, tok0:tok0 + P],
                            wv[:, 2 * cj:2 * cj + 2,
                               half * 512:(half + 1) * 512],
                            start=(cj == 0), stop=False,
                            perf_mode=DR,
                        )
                    nc.tensor.matmul(
                        ps, ones_r1[0:1, 0:P],
                        bv[0:1, half * 512:(half + 1) * 512],
                        start=False, stop=True,
                    )
                    dst = v_aug[:, it * 5 + c_i, half * 8:(half + 1) * 8, 0:64]
                    nc.vector.tensor_scalar_mul(
                        out=dst, in0=ps.rearrange("p (h d) -> p h d", d=64),
                        scalar1=SV / (SW * SX),
                    )

        def emit_attn_pair(at_ps, at_sb, it, pair, sums4, yr_tiles):
            """Scores, exp (fp8), attn@v (DoubleRow) for one head pair/item.
            Softmax sums ride row 64 of the y PSUM (ones-row of v_aug) and are
            gathered into `sums4` (32-aligned rows) by GPSIMD."""
            exps = [
                at_sb.tile([P, 5, EXPAD], F8, tag="exp0", name="exp0", bufs=4),
                at_sb.tile([P, 5, EXPAD], F8, tag="exp1", name="exp1", bufs=4),
            ]
            for c_i, (k0, kw) in enumerate(KCH):
                ks = slice(it * NTOK + k0, it * NTOK + k0 + kw)
                for hh in range(2):
                    rows = slice(hh * D, (hh + 1) * D)
                    ps_s = at_ps.tile(
                        [P, NTOK], F32, tag="score", name="score", bufs=3
                    )
                    for q0, qw in QT:
                        qs = slice(it * NTOK + q0, it * NTOK + q0 + qw)
                        nc.tensor.matmul(
                            ps_s[:kw, q0:q0 + qw],
                            qkT[rows, KC + pair, ks],
                            qkT[rows, pair, qs],
                            start=True, stop=True,
                        )
                    nc.scalar.activation(
                        exps[hh][:kw, c_i, 0:NTOK], ps_s[:kw, :],
                        AF.Exp, scale=0.125 / (SQ * SQ),
                    )
            for hh in range(2):
                h = 2 * pair + hh
                yr = at_sb.tile(
                    [65, NTOK], F32, tag=f"yraw{hh}", name=f"yraw{hh}", bufs=4
                )
                yr_tiles[(it, hh)] = yr
                # chunk-outer / query-tile-inner: both query tiles reuse each
                # chunk's stationary v load (the 65-wide tail chain is
                # otherwise LDWEIGHTS-bound)
                ps_list = [
                    at_ps.tile(
                        [P, 512], F32, tag="y", name="y", bufs=2
                    )[:65, :qw]
                    for q0, qw in QT
                ]
                for c_i, (k0, kw) in enumerate(KCH):
                    for qi, (q0, qw) in enumerate(QT):
                        nc.tensor.matmul(
                            ps_list[qi],
                            v_aug[:kw, it * 5 + c_i, h, :],
                            exps[hh][:kw, c_i, q0:q0 + qw],
                            start=(c_i == 0), stop=(c_i == 4),
                        )
                for qi, (q0, qw) in enumerate(QT):
                    nc.vector.tensor_copy(
                        out=yr[:, q0:q0 + qw], in_=ps_list[qi]
                    )
                base = 32 * (2 * it + hh)
                nc.vector.tensor_copy(
                    out=sums4[base:base + 1, :], in_=yr[64:65, :]
                )

        def finish_group(at_ps, pair, sums4, rec4, recbf, yr_tiles):
            nc.vector.reciprocal_approx_fast(out=rec4, in_=sums4)
            nc.vector.tensor_copy(out=recbf, in_=rec4)
            for it in range(2):
                for hh in range(2):
                    base = 32 * (2 * it + hh)
                    yr = yr_tiles[(it, hh)]
                    for q0, qw in QT:
                        qs = slice(it * NTOK + q0, it * NTOK + q0 + qw)
                        ps_rb = at_ps.tile(
                            [P, 512], F32, tag="y", name="rb", bufs=2
                        )[:D, :qw]
                        nc.tensor.matmul(
                            ps_rb,
                            c64[base:base + 1, 0:D],
                            recbf[base:base + 1, q0:q0 + qw],
                            start=True, stop=True,
                            tile_position=(base, 0),
                        )
                        nc.vector.tensor_tensor(
                            out=yT8[hh * D:(hh + 1) * D, pair, qs],
                            in0=yr[0:D, q0:q0 + qw],
                            in1=ps_rb, op=ALU.mult,
                        )

        # --- front: LN1, then ALL of qkv for both items ---
        with ExitStack() as s1x:
            xn_p = s1x.enter_context(tc.tile_pool(name="xn1", bufs=1))
            wqk_pool = s1x.enter_context(tc.tile_pool(name="wqkp", bufs=4))
            ln_sb = s1x.enter_context(tc.tile_pool(name="ln1sb", bufs=2))
            ln_ps = s1x.enter_context(
                tc.tile_pool(name="ln1ps", bufs=1, space="PSUM")
            )
            wv_pool = s1x.enter_context(tc.tile_pool(name="wvp", bufs=1))
            mm_ps = s1x.enter_context(
                tc.tile_pool(name="qkvps", bufs=2, space="PSUM")
            )
            xn8 = xn_p.tile([P, KC, TPAD], F8)
            nc.vector.memset(xn8[:, :, T:TPAD], 0.0)
            wv = wv_pool.tile([P, KC, C], F8)
            nc.sync.dma_start(out=wv, in_=wv_d[:])

            for t0, tw in TT:
                layernorm_tile(xT, gb1, g1c, xn8, t0, tw, ln_sb, ln_ps)
            for m in range(2 * KC):
                emit_qk(mm_ps, m, TT)
            emit_v(mm_ps, wv, 0)
            emit_v(mm_ps, wv, 1)

        # --- attention: both items' 8 head-pairs, deep-pipelined ---
        at_sb = attn_scope.enter_context(tc.tile_pool(name="atsb", bufs=2))
        wp_pool = attn_scope.enter_context(tc.tile_pool(name="wppool", bufs=1))
        wp = wp_pool.tile([P, KC, C], F8)
        nc.sync.dma_start(out=wp, in_=wp_d[:])
        with ExitStack() as w1x:
            at_ps = w1x.enter_context(
                tc.tile_pool(name="atps", bufs=1, space="PSUM")
            )
            pending = None
            for pair in range(H // 2):
                sums4 = at_sb.tile(
                    [97, NTOK], F32, tag="sums4", name="sums4", bufs=2
                )
                nc.vector.memset(sums4, 1.0)
                rec4 = at_sb.tile(
                    [97, NTOK], F32, tag="rec4", name="rec4", bufs=2
                )
                recbf = at_sb.tile(
                    [97, NTOK], BF16, tag="recbf", name="recbf", bufs=2
                )
                yr_tiles = {}
                emit_attn_pair(at_ps, at_sb, 0, pair, sums4, yr_tiles)
                if pending is not None:
                    finish_group(at_ps, *pending)
                emit_attn_pair(at_ps, at_sb, 1, pair, sums4, yr_tiles)
                pending = (pair, sums4, rec4, recbf, yr_tiles)
            finish_group(at_ps, *pending)
            # proj chains draw from the attention pool's score tag (3 bufs
            # of slack): no pool-close barrier, so they backfill the
            # ACT-bound attention tail (chain cj needs only head-pairs
            # 2cj,2cj+1 finished)
            for t0, tw in TT:
                ts = slice(t0, t0 + tw)
                for m in range(KC):
                    ps = at_ps.tile(
                        [P, NTOK], F32, tag="score", name="z", bufs=3
                    )[:, :tw]
                    for cj in range(KC // 2):
                        nc.tensor.matmul(
                            ps, wp[:, 2 * cj:2 * cj + 2, m * P:(m + 1) * P],
                            yT8[:, 2 * cj:2 * cj + 2, ts],
                            start=(cj == 0), stop=(cj == KC // 2 - 1),
                            perf_mode=DR,
                        )
                    # x2 = (z/(SW*SY) + pb) + x   (in place into xT)
                    nc.vector.affine_then_add(
                        out=xT[:, m, ts], in0=ps, in1=xT[:, m, ts],
                        scale=1.0 / (SW * SY), bias=pb[:, m:m + 1],
                    )
        attn_scope.close()  # release qkT / v_aug / exps space before MLP

        # --- proj + residual + LN2, tile-pipelined ---
        with ExitStack() as pjx:
            ln_sb2 = pjx.enter_context(tc.tile_pool(name="ln2sb", bufs=2))
            ln_ps2 = pjx.enter_context(
                tc.tile_pool(name="ln2ps", bufs=1, space="PSUM")
            )
            xn2_p = pjx.enter_context(tc.tile_pool(name="xn2", bufs=1))
            xs8 = xn2_p.tile([P, KC, TPAD], F8)

            for t0, tw in TT:
                layernorm_tile(xT, gb2, g2c, xs8, t0, tw, ln_sb2, ln_ps2)

            # ------------------------- MLP -------------------------
            h_p = pjx.enter_context(tc.tile_pool(name="hpool", bufs=1))
            hT = h_p.tile([P, MH, T], BF16)
            w2_pool = pjx.enter_context(tc.tile_pool(name="w2pool", bufs=2))
            mlp_ps = pjx.enter_context(
                tc.tile_pool(name="mlpps", bufs=2, space="PSUM")
            )
            with ExitStack() as f1x:
                w1_pool = f1x.enter_context(tc.tile_pool(name="w1pool", bufs=3))
                for mh in range(MH):
                    w1b = w1_pool.tile(
                        [P, KC, P], F8 if F8_FC1 else BF16,
                        tag="w1b", name="w1b",
                    )
                    nc.sync.dma_start(out=w1b, in_=w1_d[mh])
                    for t0, tw in TT:
                        ts = slice(t0, t0 + tw)
                        ps = mlp_ps.tile([P, 512], F32, tag="h", name="h")[:, :tw]
                        if F8_FC1:
                            for cj in range(KC // 2):
                                nc.tensor.matmul(
                                    ps, w1b[:, 2 * cj:2 * cj + 2, :],
                                    xs8[:, 2 * cj:2 * cj + 2, ts],
                                    start=(cj == 0), stop=(cj == KC // 2 - 1),
                                    perf_mode=DR,
                                )
                        else:
                            for ci in range(KC):
                                nc.tensor.matmul(
                                    ps, w1b[:, ci, :], xs8[:, ci, ts],
                                    start=(ci == 0), stop=(ci == KC - 1),
                                )
                        nc.scalar.activation(
                            hT[:, mh, ts], ps, AF.Gelu,
                            bias=b1[:, mh:mh + 1], scale=1.0 / (SW * SX),
                        )

            with ExitStack() as f2x:
                o_pool = f2x.enter_context(tc.tile_pool(name="opool", bufs=3))
                for m in range(KC):
                    w2b = w2_pool.tile([P, MH, P], BF16, tag="w2b", name="w2b")
                    nc.sync.dma_start(out=w2b, in_=w2_d[m])
                    for t0, tw in TT:
                        ts = slice(t0, t0 + tw)
                        ps = mlp_ps.tile([P, 512], F32, tag="h", name="o")[:, :tw]
                        for kh in range(MH):
                            nc.tensor.matmul(
                                ps, w2b[:, kh, :], hT[:, kh, ts],
                                start=(kh == 0), stop=(kh == MH - 1),
                            )
                        osb = o_pool.tile([P, 512], F32, tag="osb",
                                          name="osb")[:, :tw]
                        nc.vector.scalar_tensor_tensor(
                            osb, ps, b2[:, m:m + 1], xT[:, m, ts],
                            ALU.add, ALU.add,
                        )
                        nc.sync.dma_start(out=out_d[:, m, ts], in_=osb)

    nc.compile()
    return nc


def _program():
    global _NC
    if _NC is None:
        _NC = _build()
    return _NC


def _prep_inputs(x, w_qkv, b_qkv, w_proj, b_proj, ln1_g, ln1_b, ln2_g, ln2_b,
                 w_fc1, b_fc1, w_fc2, b_fc2):
    bf = ml_dtypes.bfloat16
    f8 = ml_dtypes.float8_e4m3

    def to8(a, s):
        return np.clip(
            np.asarray(a, dtype=np.float32) * s, -240.0, 240.0
        ).astype(f8)

    x = np.asarray(x, dtype=np.float32)
    B = x.shape[0]

    # feature-major x, chunked: [B, P, KC, NTOK]
    xTt = np.ascontiguousarray(
        x.transpose(0, 2, 1).reshape(B, KC, P, NTOK).transpose(0, 2, 1, 3)
    )

    wqkT = w_qkv[: 2 * C].T.reshape(KC, P, 2 * KC, P).transpose(2, 1, 0, 3)
    wqk8 = to8(np.ascontiguousarray(wqkT), SW)
    wv8 = to8(np.ascontiguousarray(
        w_qkv[2 * C:].T.reshape(KC, P, C).transpose(1, 0, 2)), SW)
    wp8 = to8(np.ascontiguousarray(
        w_proj.T.reshape(KC, P, C).transpose(1, 0, 2)), SW)
    w1T = w_fc1.T.reshape(KC, P, MH, P).transpose(2, 1, 0, 3)
    w1T = np.ascontiguousarray(w1T)
    w18 = to8(w1T, SW) if F8_FC1 else w1T.astype(bf)
    w2T = w_fc2.T.reshape(MH, P, KC, P).transpose(2, 1, 0, 3)
    w2T = np.ascontiguousarray(w2T).astype(bf)

    qkb = np.ascontiguousarray(
        b_qkv[: 2 * C].reshape(2 * KC, P).T * SQ).astype(np.float32)
    bv = np.ascontiguousarray(
        b_qkv[2 * C:].reshape(1, C) * (SW * SX)).astype(bf)
    pb = np.ascontiguousarray(b_proj.reshape(KC, P).T).astype(np.float32)
    b1a = np.ascontiguousarray(b_fc1.reshape(MH, P).T).astype(np.float32)
    b2a = np.ascontiguousarray(b_fc2.reshape(KC, P).T).astype(np.float32)
    gb1 = np.ascontiguousarray(
        np.stack([ln1_g, ln1_b]).reshape(2, KC, P) * SX).astype(bf)
    gb2 = np.ascontiguousarray(
        np.stack([ln2_g, ln2_b]).reshape(2, KC, P) * SX).astype(bf)
    g1c = np.ascontiguousarray(
        ln1_g.reshape(KC, P).T * SX).astype(np.float32)
    g2c = np.ascontiguousarray(
        ln2_g.reshape(KC, P).T * SX).astype(np.float32)

    shared = dict(
        wqk=wqk8, wv=wv8, wp=wp8, w1=w18, w2=w2T, qkb=qkb, bv=bv, pb=pb,
        b1=b1a, b2=b2a, gb1=gb1, gb2=gb2, g1c=g1c, g2c=g2c,
    )
    in_maps = []
    for core in range(B // 2):
        xc = np.concatenate([xTt[2 * core], xTt[2 * core + 1]], axis=2)
        in_maps.append(dict(xT=np.ascontiguousarray(xc), **shared))
    return in_maps


def kernel(x, w_qkv, b_qkv, w_proj, b_proj, ln1_g, ln1_b, ln2_g, ln2_b,
           w_fc1, b_fc1, w_fc2, b_fc2, _trace=False, _tmpdir=None):
    global LAST_EXEC_NS
    B = np.asarray(x).shape[0]
    in_maps = _prep_inputs(
        x, w_qkv, b_qkv, w_proj, b_proj, ln1_g, ln1_b, ln2_g, ln2_b,
        w_fc1, b_fc1, w_fc2, b_fc2,
    )

    nc = _program()
    res = run_bass_kernel_spmd(
        nc, in_maps, list(range(N_CORES)), trace=_trace, tmpdir=_tmpdir
    )
    LAST_EXEC_NS = res.exec_time_ns

    out = np.empty((B, NTOK, C), dtype=np.float32)
    for core in range(N_CORES):
        o = res.results[core]["outT"]  # [P, KC, T]
        full = o.transpose(1, 0, 2).reshape(C, T)
        out[2 * core] = full[:, :NTOK].T
        out[2 * core + 1] = full[:, NTOK:].T
    return out


# revision 31
# speedup vs baseline: 1.1918x; 1.1918x over previous
"""Trainium2 Bass kernel for a ViT-style dense transformer block (v2, fp8).

Reference computation:
    xn = LN1(x); qkv = xn @ Wqkv.T + b; 16-head softmax attention;
    x = x + attn_out @ Wp.T + bp;
    out = x + gelu(LN2(x) @ W1.T + b1) @ W2.T + b2
Shapes: x [16, 577, 1024], heads 16, head_dim 64, hidden 4096.

Sharding: data-parallel over batch across 8 NeuronCores; each core gets 2
batch items concatenated along tokens (T = 2*577 = 1154). No collectives.

v2/v3 changes vs the bf16 baseline (742us -> ~581us measured):
  * qkv / v / proj / fc1 matmuls run in fp8(e4m3) DoubleRow mode
    (2 contraction chunks per instruction, ~1.4x tensor throughput); fc2
    stays bf16 (fp8 there would blow the 2e-2 error budget - fc1+fc2
    together measured 2.4e-2 in simulation, fc1 alone 1.6e-2, qkv/proj
    each < 2e-3).  Attention scores and attn@v run in fp8 normal mode
    (DoubleRow with the odd 65-partition output of attn@v produced NaNs
    on hardware; normal-mode fp8 is safe).
  * Activations for those matmuls are stored as fp8 with power-of-two
    scales (xn*32, q/k*32, v*32, probs, y*64); scales are folded into the
    PSUM-evacuation activation scale and the ones/bias rank-1 rows.
  * Attention softmax: the denominator comes from an appended ones-row in
    v_aug (as before) but is evacuated together with y in a single [65, qw]
    copy; the 4 sums rows of a head-pair group are gathered by GPSIMD into
    one 32-row-aligned tile, inverted with reciprocal_approx_fast (one DVE
    op instead of an 8-cycle/element exact reciprocal), broadcast by a
    rank-1 matmul of a 64-valued column (folding the y fp8 scale), and
    multiplied in.
  * LayerNorm: squares and PSUM->SBUF row copies moved to the scalar
    engine, the x*g*rstd pass split between GPSIMD and DVE, rstd via
    reciprocal_approx_fast. LN2 is emitted per token-tile interleaved with
    the proj evacuations so fc1 starts while proj finishes.
"""

import numpy as np
import ml_dtypes
from contextlib import ExitStack

import concourse.bacc as bacc
import concourse.mybir as mybir
import concourse.tile as tile
from concourse.bass_utils import run_bass_kernel_spmd

F32 = mybir.dt.float32
BF16 = mybir.dt.bfloat16
F8 = mybir.dt.float8e4
AF = mybir.ActivationFunctionType
ALU = mybir.AluOpType
DR = mybir.MatmulPerfMode.DoubleRow

N_CORES = 8
P = 128
C = 1024
KC = C // P          # 8 feature chunks
H = 16               # heads
D = 64               # head dim
HID = 4096
MH = HID // P        # 32 hidden chunks
NTOK = 577           # tokens per item
T = 2 * NTOK         # tokens per core
TPAD = 1280          # padded fp8 activation free dim (%16, covers v overread)
EXPAD = 592          # padded exp-tile free dim (%16)
EPS = 1e-5

SX = 32.0            # fp8 scale: xn / xs (LN outputs)
SW = 1024.0          # fp8 scale: weights
SQ = 32.0            # fp8 scale: q / k
SV = 32.0            # fp8 scale: v (also the ones-row value in v_aug)
SY = 64.0            # fp8 scale: yT (bcast column value)

F8_FC1 = True        # fc1 in fp8 DoubleRow (rel-err budget allows one fc)

# token tiles for dense (non-attention) phases; <=512 for one PSUM bank
TT = [(0, 512), (512, 512), (1024, 130)]
# query tiles per item (relative to item start)
QT = [(0, 512), (512, 65)]
# key chunks per item: 4 full 128s + one 65
KCH = [(0, 128), (128, 128), (256, 128), (384, 128), (512, 65)]

LAST_EXEC_NS = None
_NC = None


def _build():
    nc = bacc.Bacc(
        "TRN2", target_bir_lowering=False, debug=False, num_devices=N_CORES
    )

    xT_d = nc.dram_tensor("xT", [P, KC, T], F32, kind="ExternalInput")
    wqk_d = nc.dram_tensor("wqk", [2 * KC, P, KC, P], F8, kind="ExternalInput")
    wv_d = nc.dram_tensor("wv", [P, KC, C], F8, kind="ExternalInput")
    wp_d = nc.dram_tensor("wp", [P, KC, C], F8, kind="ExternalInput")
    w1_d = nc.dram_tensor(
        "w1", [MH, P, KC, P], F8 if F8_FC1 else BF16, kind="ExternalInput"
    )
    w2_d = nc.dram_tensor("w2", [KC, P, MH, P], BF16, kind="ExternalInput")
    qkb_d = nc.dram_tensor("qkb", [P, 2 * KC], F32, kind="ExternalInput")
    bv_d = nc.dram_tensor("bv", [1, C], BF16, kind="ExternalInput")
    pb_d = nc.dram_tensor("pb", [P, KC], F32, kind="ExternalInput")
    b1_d = nc.dram_tensor("b1", [P, MH], F32, kind="ExternalInput")
    b2_d = nc.dram_tensor("b2", [P, KC], F32, kind="ExternalInput")
    gb1_d = nc.dram_tensor("gb1", [2, KC, P], BF16, kind="ExternalInput")
    gb2_d = nc.dram_tensor("gb2", [2, KC, P], BF16, kind="ExternalInput")
    g1c_d = nc.dram_tensor("g1c", [P, KC], F32, kind="ExternalInput")
    g2c_d = nc.dram_tensor("g2c", [P, KC], F32, kind="ExternalInput")
    out_d = nc.dram_tensor("outT", [P, KC, T], F32, kind="ExternalOutput")

    with tile.TileContext(nc) as tc, ExitStack() as top:
        const_p = top.enter_context(tc.tile_pool(name="consts", bufs=1))
        xp = top.enter_context(tc.tile_pool(name="xres", bufs=1))
        yp = top.enter_context(tc.tile_pool(name="ypool", bufs=1))

        # residual stream, feature-major f32 (updated in place with x2);
        # x pieces go out first so LN1 tile0 isn't queued behind the consts
        xT = xp.tile([P, KC, T], F32)
        for _t0, _tw in TT:
            for _c in range(0, KC, 4):
                nc.sync.dma_start(
                    out=xT[:, _c:_c + 4, _t0:_t0 + _tw],
                    in_=xT_d[:, _c:_c + 4, _t0:_t0 + _tw],
                )

        qkb = const_p.tile([P, 2 * KC], F32)
        nc.sync.dma_start(out=qkb, in_=qkb_d[:])
        bv = const_p.tile([1, C], BF16)
        nc.sync.dma_start(out=bv, in_=bv_d[:])
        pb = const_p.tile([P, KC], F32)
        nc.sync.dma_start(out=pb, in_=pb_d[:])
        b1 = const_p.tile([P, MH], F32)
        nc.sync.dma_start(out=b1, in_=b1_d[:])
        b2 = const_p.tile([P, KC], F32)
        nc.sync.dma_start(out=b2, in_=b2_d[:])
        gb1 = const_p.tile([2, KC, P], BF16)
        nc.sync.dma_start(out=gb1, in_=gb1_d[:])
        gb2 = const_p.tile([2, KC, P], BF16)
        nc.sync.dma_start(out=gb2, in_=gb2_d[:])
        g1c = const_p.tile([P, KC], F32)
        nc.sync.dma_start(out=g1c, in_=g1c_d[:])
        g2c = const_p.tile([P, KC], F32)
        nc.sync.dma_start(out=g2c, in_=g2c_d[:])
        inv_ones = const_p.tile([P, 1], BF16)
        nc.vector.memset(inv_ones, 1.0 / C)
        ones_pp = const_p.tile([P, P], BF16)
        nc.vector.memset(ones_pp, 1.0)
        ones_r1 = ones_pp[0:1, :]
        c64 = const_p.tile([P, D], BF16)  # SY-valued col; rank-1 lhsT
        nc.vector.memset(c64, SY)

        def layernorm_tile(src, gb, g_c, dst, t0, tw, ln_sb, ln_ps):
            """dst[:, :, t0:t0+tw] (fp8) = SX * LN(src[f32]), one token tile."""
            ts = slice(t0, t0 + tw)
            rows2 = ln_sb.tile([2, 512], BF16, tag="rows2", name="rows2",
                               bufs=2)[:, :tw]
            nc.vector.memset(rows2, -1.0)  # row1 stays -1; row0 overwritten
            s1 = ln_ps.tile([1, 512], F32, tag="s1", name="s1", bufs=1)[:, :tw]
            s2 = ln_ps.tile([1, 512], F32, tag="s2", name="s2", bufs=1)[:, :tw]
            for ci in range(KC):
                xbf = ln_sb.tile([P, 512], BF16, tag="xbf", name="xbf",
                                 bufs=2)[:, :tw]
                nc.scalar.copy(xbf, src[:, ci, ts])
                xsq = ln_sb.tile([P, 512], BF16, tag="xsq", name="xsq",
                                 bufs=2)[:, :tw]
                nc.scalar.square(xsq, xbf)
                nc.tensor.matmul(
                    s1, inv_ones, xbf, start=(ci == 0), stop=(ci == KC - 1)
                )
                nc.tensor.matmul(
                    s2, inv_ones, xsq, start=(ci == 0), stop=(ci == KC - 1)
                )
            msq = ln_sb.tile([1, 512], F32, tag="msq", name="msq", bufs=1)[:, :tw]
            nc.scalar.square(msq, s1)
            veps = ln_sb.tile([1, 512], F32, tag="veps", name="veps", bufs=1)[:, :tw]
            nc.vector.scalar_tensor_tensor(
                veps, s2, EPS, msq, ALU.add, ALU.subtract
            )
            sd = ln_sb.tile([1, 512], F32, tag="sd", name="sd", bufs=1)[:, :tw]
            nc.scalar.sqrt(sd, veps)
            r32 = ln_sb.tile([1, 512], F32, tag="r32", name="r32", bufs=1)[:, :tw]
            nc.vector.reciprocal_approx_fast(out=r32, in_=sd)
            rbf = ln_sb.tile([1, 512], BF16, tag="rbf", name="rbf", bufs=1)[:, :tw]
            nc.vector.tensor_copy(out=rbf, in_=r32)
            nc.vector.tensor_mul(rows2[0:1, :], s1, r32)  # mr row
            ps_rb = ln_ps.tile([P, 512], F32, tag="lnrb", name="lnrb",
                               bufs=1)[:, :tw]
            nc.tensor.matmul(ps_rb, ones_r1, rbf, start=True, stop=True)
            rb = ln_sb.tile([P, 512], F32, tag="rb", name="rb", bufs=2)[:, :tw]
            nc.scalar.copy(rb, ps_rb)
            for ci in range(KC):
                # gmrb[c,t] = SX*(g[c]*mr[t] - b[c])
                gmrb = ln_ps.tile([P, 512], F32, tag="gmrb", name="gmrb",
                                  bufs=1)[:, :tw]
                nc.tensor.matmul(
                    gmrb, gb[:, ci, :], rows2, start=True, stop=True
                )
                t1 = ln_sb.tile([P, 512], F32, tag="t1", name="t1",
                                bufs=3)[:, :tw]
                eng = nc.gpsimd if ci < 4 else nc.vector
                eng.tensor_tensor(
                    out=t1, in0=src[:, ci, ts], in1=rb, op=ALU.mult
                )
                nc.vector.scalar_tensor_tensor(
                    dst[:, ci, ts], t1, g_c[:, ci:ci + 1], gmrb,
                    ALU.mult, ALU.subtract,
                )

        # --------------- LN1 + QKV (fp8 DoubleRow) -----------------------
        attn_scope = top.enter_context(ExitStack())
        qk_p = attn_scope.enter_context(tc.tile_pool(name="qkp", bufs=1))
        v_p = attn_scope.enter_context(tc.tile_pool(name="vp", bufs=1))
        yT8 = yp.tile([P, KC, TPAD], F8)  # attention output (pre-proj), *SY
        qkT = qk_p.tile([P, 2 * KC, T], F8)  # q,k *SQ
        v_aug = v_p.tile([P, 10, H, 65], F8)  # v *SV, ones-row = SV
        nc.vector.memset(v_aug[:, :, :, 64:65], SV)
        xn8 = None
        wqk_pool = None

        def emit_qk(mm_ps, m, tiles, psbufs=2):
            wqkb = wqk_pool.tile([P, KC, P], F8, tag="wqkb", name="wqkb")
            nc.sync.dma_start(out=wqkb, in_=wqk_d[m])
            for t0, tw in tiles:
                ts = slice(t0, t0 + tw)
                ps = mm_ps.tile(
                    [P, 512], F32, tag="qk", name="qk", bufs=psbufs
                )[:, :tw]
                for cj in range(KC // 2):
                    nc.tensor.matmul(
                        ps, wqkb[:, 2 * cj:2 * cj + 2, :],
                        xn8[:, 2 * cj:2 * cj + 2, ts],
                        start=(cj == 0), stop=(cj == KC // 2 - 1),
                        perf_mode=DR,
                    )
                nc.scalar.activation(
                    qkT[:, m, ts], ps, AF.Identity,
                    bias=qkb[:, m:m + 1], scale=SQ / (SW * SX),
                )

        def emit_v(mm_ps, wv, it):
            for c_i, (k0, kw) in enumerate(KCH):
                tok0 = it * NTOK + k0
                for half in range(2):
                    ps = mm_ps.tile([P, 512], F32, tag="v", name="v", bufs=2)
                    for cj in range(KC // 2):
                        nc.tensor.matmul(
                            ps,
                            xn8[:, 2 * cj:2 * cj + 2, tok0:tok0 + P],
                            wv[:, 2 * cj:2 * cj + 2,
                               half * 512:(half + 1) * 512],
                            start=(cj == 0), stop=False,
                            perf_mode=DR,
                        )
                    nc.tensor.matmul(
                        ps, ones_r1[0:1, 0:P],
                        bv[0:1, half * 512:(half + 1) * 512],
                        start=False, stop=True,
                    )
                    dst = v_aug[:, it * 5 + c_i, half * 8:(half + 1) * 8, 0:64]
                    nc.vector.tensor_scalar_mul(
                        out=dst, in0=ps.rearrange("p (h d) -> p h d", d=64),
                        scalar1=SV / (SW * SX),
                    )

        def emit_attn_pair(at_ps, at_sb, it, pair, sums4, yr_tiles):
            """Scores, exp (fp8), attn@v (DoubleRow) for one head pair/item.
            Softmax sums ride row 64 of the y PSUM (ones-row of v_aug) and are
            gathered into `sums4` (32-aligned rows) by GPSIMD."""
            exps = [
                at_sb.tile([P, 5, EXPAD], F8, tag="exp0", name="exp0", bufs=4),
                at_sb.tile([P, 5, EXPAD], F8, tag="exp1", name="exp1", bufs=4),
            ]
            for c_i, (k0, kw) in enumerate(KCH):
                ks = slice(it * NTOK + k0, it * NTOK + k0 + kw)
                for hh in range(2):
                    rows = slice(hh * D, (hh + 1) * D)
                    ps_s = at_ps.tile(
                        [P, NTOK], F32, tag="score", name="score", bufs=3
                    )
                    for q0, qw in QT:
                        qs = slice(it * NTOK + q0, it * NTOK + q0 + qw)
                        nc.tensor.matmul(
                            ps_s[:kw, q0:q0 + qw],
                            qkT[rows, KC + pair, ks],
                            qkT[rows, pair, qs],
                            start=True, stop=True,
                        )
                    nc.scalar.activation(
                        exps[hh][:kw, c_i, 0:NTOK], ps_s[:kw, :],
                        AF.Exp, scale=0.125 / (SQ * SQ),
                    )
            for hh in range(2):
                h = 2 * pair + hh
                yr = at_sb.tile(
                    [65, NTOK], F32, tag=f"yraw{hh}", name=f"yraw{hh}", bufs=4
                )
                yr_tiles[(it, hh)] = yr
                # chunk-outer / query-tile-inner: both query tiles reuse each
                # chunk's stationary v load (the 65-wide tail chain is
                # otherwise LDWEIGHTS-bound)
                ps_list = [
                    at_ps.tile(
                        [P, 512], F32, tag="y", name="y", bufs=2
                    )[:65, :qw]
                    for q0, qw in QT
                ]
                for c_i, (k0, kw) in enumerate(KCH):
                    for qi, (q0, qw) in enumerate(QT):
                        nc.tensor.matmul(
                            ps_list[qi],
                            v_aug[:kw, it * 5 + c_i, h, :],
                            exps[hh][:kw, c_i, q0:q0 + qw],
                            start=(c_i == 0), stop=(c_i == 4),
                        )
                for qi, (q0, qw) in enumerate(QT):
                    nc.vector.tensor_copy(
                        out=yr[:, q0:q0 + qw], in_=ps_list[qi]
                    )
                base = 32 * (2 * it + hh)
                nc.vector.tensor_copy(
                    out=sums4[base:base + 1, :], in_=yr[64:65, :]
                )

        def finish_group(at_ps, pair, sums4, rec4, recbf, yr_tiles):
            nc.vector.reciprocal_approx_fast(out=rec4, in_=sums4)
            nc.vector.tensor_copy(out=recbf, in_=rec4)
            for it in range(2):
                for hh in range(2):
                    base = 32 * (2 * it + hh)
                    yr = yr_tiles[(it, hh)]
                    for q0, qw in QT:
                        qs = slice(it * NTOK + q0, it * NTOK + q0 + qw)
                        ps_rb = at_ps.tile(
                            [P, 512], F32, tag="y", name="rb", bufs=2
                        )[:D, :qw]
                        nc.tensor.matmul(
                            ps_rb,
                            c64[base:base + 1, 0:D],
                            recbf[base:base + 1, q0:q0 + qw],
                            start=True, stop=True,
                            tile_position=(base, 0),
                        )
                        nc.vector.tensor_tensor(
                            out=yT8[hh * D:(hh + 1) * D, pair, qs],
                            in0=yr[0:D, q0:q0 + qw],
                            in1=ps_rb, op=ALU.mult,
                        )

        # --- front: LN1, then ALL of qkv for both items ---
        with ExitStack() as s1x:
            xn_p = s1x.enter_context(tc.tile_pool(name="xn1", bufs=1))
            wqk_pool = s1x.enter_context(tc.tile_pool(name="wqkp", bufs=4))
            ln_sb = s1x.enter_context(tc.tile_pool(name="ln1sb", bufs=2))
            ln_ps = s1x.enter_context(
                tc.tile_pool(name="ln1ps", bufs=1, space="PSUM")
            )
            wv_pool = s1x.enter_context(tc.tile_pool(name="wvp", bufs=1))
            mm_ps = s1x.enter_context(
                tc.tile_pool(name="qkvps", bufs=2, space="PSUM")
            )
            xn8 = xn_p.tile([P, KC, TPAD], F8)
            nc.vector.memset(xn8[:, :, T:TPAD], 0.0)
            wv = wv_pool.tile([P, KC, C], F8)
            nc.sync.dma_start(out=wv, in_=wv_d[:])

            for t0, tw in TT:
                layernorm_tile(xT, gb1, g1c, xn8, t0, tw, ln_sb, ln_ps)
            for m in range(2 * KC):
                emit_qk(mm_ps, m, TT)
            emit_v(mm_ps, wv, 0)
            emit_v(mm_ps, wv, 1)

        # --- attention: both items' 8 head-pairs, deep-pipelined ---
        at_sb = attn_scope.enter_context(tc.tile_pool(name="atsb", bufs=2))
        wp_pool = attn_scope.enter_context(tc.tile_pool(name="wppool", bufs=1))
        wp = wp_pool.tile([P, KC, C], F8)
        nc.sync.dma_start(out=wp, in_=wp_d[:])
        with ExitStack() as w1x:
            at_ps = w1x.enter_context(
                tc.tile_pool(name="atps", bufs=1, space="PSUM")
            )
            pending = None
            for pair in range(H // 2):
                sums4 = at_sb.tile(
                    [97, NTOK], F32, tag="sums4", name="sums4", bufs=2
                )
                nc.vector.memset(sums4, 1.0)
                rec4 = at_sb.tile(
                    [97, NTOK], F32, tag="rec4", name="rec4", bufs=2
                )
                recbf = at_sb.tile(
                    [97, NTOK], BF16, tag="recbf", name="recbf", bufs=2
                )
                yr_tiles = {}
                emit_attn_pair(at_ps, at_sb, 0, pair, sums4, yr_tiles)
                if pending is not None:
                    finish_group(at_ps, *pending)
                emit_attn_pair(at_ps, at_sb, 1, pair, sums4, yr_tiles)
                pending = (pair, sums4, rec4, recbf, yr_tiles)
            finish_group(at_ps, *pending)
        attn_scope.close()  # release qkT / v_aug / exps space before MLP

        # --- proj + residual + LN2, tile-pipelined ---
        with ExitStack() as pjx:
            pj_ps = pjx.enter_context(
                tc.tile_pool(name="pjps", bufs=2, space="PSUM")
            )
            ln_sb2 = pjx.enter_context(tc.tile_pool(name="ln2sb", bufs=2))
            ln_ps2 = pjx.enter_context(
                tc.tile_pool(name="ln2ps", bufs=1, space="PSUM")
            )
            xn2_p = pjx.enter_context(tc.tile_pool(name="xn2", bufs=1))
            xs8 = xn2_p.tile([P, KC, TPAD], F8)

            for t0, tw in TT:
                ts = slice(t0, t0 + tw)
                for m in range(KC):
                    ps = pj_ps.tile(
                        [P, 512], F32, tag="z", name="z", bufs=2
                    )[:, :tw]
                    for cj in range(KC // 2):
                        nc.tensor.matmul(
                            ps, wp[:, 2 * cj:2 * cj + 2, m * P:(m + 1) * P],
                            yT8[:, 2 * cj:2 * cj + 2, ts],
                            start=(cj == 0), stop=(cj == KC // 2 - 1),
                            perf_mode=DR,
                        )
                    # x2 = (z/(SW*SY) + pb) + x   (in place into xT)
                    nc.vector.affine_then_add(
                        out=xT[:, m, ts], in0=ps, in1=xT[:, m, ts],
                        scale=1.0 / (SW * SY), bias=pb[:, m:m + 1],
                    )
                layernorm_tile(xT, gb2, g2c, xs8, t0, tw, ln_sb2, ln_ps2)

            # ------------------------- MLP -------------------------
            h_p = pjx.enter_context(tc.tile_pool(name="hpool", bufs=1))
            hT = h_p.tile([P, MH, T], BF16)
            w2_pool = pjx.enter_context(tc.tile_pool(name="w2pool", bufs=2))
            mlp_ps = pjx.enter_context(
                tc.tile_pool(name="mlpps", bufs=2, space="PSUM")
            )
            with ExitStack() as f1x:
                w1_pool = f1x.enter_context(tc.tile_pool(name="w1pool", bufs=3))
                for mh in range(MH):
                    w1b = w1_pool.tile(
                        [P, KC, P], F8 if F8_FC1 else BF16,
                        tag="w1b", name="w1b",
                    )
                    nc.sync.dma_start(out=w1b, in_=w1_d[mh])
                    for t0, tw in TT:
                        ts = slice(t0, t0 + tw)
                        ps = mlp_ps.tile([P, 512], F32, tag="h", name="h")[:, :tw]
                        if F8_FC1:
                            for cj in range(KC // 2):
                                nc.tensor.matmul(
                                    ps, w1b[:, 2 * cj:2 * cj + 2, :],
                                    xs8[:, 2 * cj:2 * cj + 2, ts],
                                    start=(cj == 0), stop=(cj == KC // 2 - 1),
                                    perf_mode=DR,
                                )
                        else:
                            for ci in range(KC):
                                nc.tensor.matmul(
                                    ps, w1b[:, ci, :], xs8[:, ci, ts],
                                    start=(ci == 0), stop=(ci == KC - 1),
                                )
                        nc.scalar.activation(
                            hT[:, mh, ts], ps, AF.Gelu,
                            bias=b1[:, mh:mh + 1], scale=1.0 / (SW * SX),
                        )

            with ExitStack() as f2x:
                o_pool = f2x.enter_context(tc.tile_pool(name="opool", bufs=3))
                for m in range(KC):
                    w2b = w2_pool.tile([P, MH, P], BF16, tag="w2b", name="w2b")
                    nc.sync.dma_start(out=w2b, in_=w2_d[m])
                    for t0, tw in TT:
                        ts = slice(t0, t0 + tw)
                        ps = mlp_ps.tile([P, 512], F32, tag="h", name="o")[:, :tw]
                        for kh in range(MH):
                            nc.tensor.matmul(
                                ps, w2b[:, kh, :], hT[:, kh, ts],
                                start=(kh == 0), stop=(kh == MH - 1),
                            )
                        osb = o_pool.tile([P, 512], F32, tag="osb",
                                          name="osb")[:, :tw]
                        nc.vector.scalar_tensor_tensor(
                            osb, ps, b2[:, m:m + 1], xT[:, m, ts],
                            ALU.add, ALU.add,
                        )
                        nc.sync.dma_start(out=out_d[:, m, ts], in_=osb)

    nc.compile()
    return nc


def _program():
    global _NC
    if _NC is None:
        _NC = _build()
    return _NC


def _prep_inputs(x, w_qkv, b_qkv, w_proj, b_proj, ln1_g, ln1_b, ln2_g, ln2_b,
                 w_fc1, b_fc1, w_fc2, b_fc2):
    bf = ml_dtypes.bfloat16
    f8 = ml_dtypes.float8_e4m3

    def to8(a, s):
        return np.clip(
            np.asarray(a, dtype=np.float32) * s, -240.0, 240.0
        ).astype(f8)

    x = np.asarray(x, dtype=np.float32)
    B = x.shape[0]

    # feature-major x, chunked: [B, P, KC, NTOK]
    xTt = np.ascontiguousarray(
        x.transpose(0, 2, 1).reshape(B, KC, P, NTOK).transpose(0, 2, 1, 3)
    )

    wqkT = w_qkv[: 2 * C].T.reshape(KC, P, 2 * KC, P).transpose(2, 1, 0, 3)
    wqk8 = to8(np.ascontiguousarray(wqkT), SW)
    wv8 = to8(np.ascontiguousarray(
        w_qkv[2 * C:].T.reshape(KC, P, C).transpose(1, 0, 2)), SW)
    wp8 = to8(np.ascontiguousarray(
        w_proj.T.reshape(KC, P, C).transpose(1, 0, 2)), SW)
    w1T = w_fc1.T.reshape(KC, P, MH, P).transpose(2, 1, 0, 3)
    w1T = np.ascontiguousarray(w1T)
    w18 = to8(w1T, SW) if F8_FC1 else w1T.astype(bf)
    w2T = w_fc2.T.reshape(MH, P, KC, P).transpose(2, 1, 0, 3)
    w2T = np.ascontiguousarray(w2T).astype(bf)

    qkb = np.ascontiguousarray(
        b_qkv[: 2 * C].reshape(2 * KC, P).T * SQ).astype(np.float32)
    bv = np.ascontiguousarray(
        b_qkv[2 * C:].reshape(1, C) * (SW * SX)).astype(bf)
    pb = np.ascontiguousarray(b_proj.reshape(KC, P).T).astype(np.float32)
    b1a = np.ascontiguousarray(b_fc1.reshape(MH, P).T).astype(np.float32)
    b2a = np.ascontiguousarray(b_fc2.reshape(KC, P).T).astype(np.float32)
    gb1 = np.ascontiguousarray(
        np.stack([ln1_g, ln1_b]).reshape(2, KC, P) * SX).astype(bf)
    gb2 = np.ascontiguousarray(
        np.stack([ln2_g, ln2_b]).reshape(2, KC, P) * SX).astype(bf)
    g1c = np.ascontiguousarray(
        ln1_g.reshape(KC, P).T * SX).astype(np.float32)
    g2c = np.ascontiguousarray(
        ln2_g.reshape(KC, P).T * SX).astype(np.float32)

    shared = dict(
        wqk=wqk8, wv=wv8, wp=wp8, w1=w18, w2=w2T, qkb=qkb, bv=bv, pb=pb,
        b1=b1a, b2=b2a, gb1=gb1, gb2=gb2, g1c=g1c, g2c=g2c,
    )
    in_maps = []
    for core in range(B // 2):
        xc = np.concatenate([xTt[2 * core], xTt[2 * core + 1]], axis=2)
        in_maps.append(dict(xT=np.ascontiguousarray(xc), **shared))
    return in_maps


def kernel(x, w_qkv, b_qkv, w_proj, b_proj, ln1_g, ln1_b, ln2_g, ln2_b,
           w_fc1, b_fc1, w_fc2, b_fc2, _trace=False, _tmpdir=None):
    global LAST_EXEC_NS
    B = np.asarray(x).shape[0]
    in_maps = _prep_inputs(
        x, w_qkv, b_qkv, w_proj, b_proj, ln1_g, ln1_b, ln2_g, ln2_b,
        w_fc1, b_fc1, w_fc2, b_fc2,
    )

    nc = _program()
    res = run_bass_kernel_spmd(
        nc, in_maps, list(range(N_CORES)), trace=_trace, tmpdir=_tmpdir
    )
    LAST_EXEC_NS = res.exec_time_ns

    out = np.empty((B, NTOK, C), dtype=np.float32)
    for core in range(N_CORES):
        o = res.results[core]["outT"]  # [P, KC, T]
        full = o.transpose(1, 0, 2).reshape(C, T)
        out[2 * core] = full[:, :NTOK].T
        out[2 * core + 1] = full[:, NTOK:].T
    return out
